# revision 54
# baseline (speedup 1.0000x reference)
"""NeuralMemory (scatter_memory) Trainium2 Bass kernel, 8-core SPMD.

Strategy (v2 — overlap-focused rewrite):
  Phase A (data-parallel over all B*T tokens, 528/core, unpadded):
    project k/v/alr, 2-layer ResLinear forward + manual backward in
    feature-major layout with fp32r matmuls, PE-transpose the four dW
    operands into token-major (bf16), per-core partial dW^T.
  The dW AllReduce is SPLIT: dW1 reduces while the dW0 backward tail
    still computes; dW0 reduces while phase-C q-projection + weight
    updates run.  AdamW first step reduces to
    w_new = w*(1-lr*wd) - lr*sign(g), identical on every core.
  Phase C (each core owns one (batch, 512-token output range)): recompute
    queries + retrieval over own tokens + 512-token halo (padded to 1024),
    sliding-window attention in bf16 with relative-position triangle masks
    and an additive key-validity bias; softmax denominator broadcast via a
    gpsimd partition_broadcast (no DRAM round trip); output projection.
  All phase-C inputs stream in on the gpsimd DMA queue at kernel start.
"""
import numpy as np
import concourse.bass as bass
import concourse.tile as tile
import concourse.mybir as mybir
from concourse import bass_utils
import bass_rust

F32 = mybir.dt.float32
BF16 = mybir.dt.bfloat16
F32R = mybir.dt.float32r
AF = mybir.ActivationFunctionType
OP = mybir.AluOpType

NCORES = 8
B, S, D = 2, 2048, 512
M, C, H, WIN = 64, 16, 8, 512
N_LAYERS = 2
MAX_ALR = 0.01
LR, WD, EPS = 1e-3, 1e-2, 1e-8
T = M + S                  # 2112
NTOK = B * T               # 4224
TA = NTOK // NCORES        # 528 tokens/core in phase A (no padding)
TC = 1024                  # phase-C halo+own width (8 x 128)
DT = D // 128              # 4 feature tiles
HD = D // H                # 64
NTT = 5                    # phase-A token tiles
TTW = [128, 128, 128, 128, 16]
HALVES = ((0, 264), (264, 264))


def split_waits(nc):
    """This walrus build encodes at most ONE sync wait per instruction.
    Hoist excess waits onto injected EventSemaphore instructions."""
    n = 0
    for fn in nc.m.functions:
        for blk in fn.blocks:
            newl = []
            for ins in blk.instructions:
                si = ins.sync_info
                if si is not None and len(si.on_wait) > 1:
                    waits = list(si.on_wait)
                    for w in waits[:-1]:
                        ev = mybir.InstEventSemaphore(
                            name=f"{ins.name}_w{n}", ins=[], outs=[])
                        ev.engine = ins.engine
                        ev.sync_info = bass_rust.SyncInfo(on_wait=[w], on_update=[])
                        newl.append(ev)
                        n += 1
                    ins.sync_info = bass_rust.SyncInfo(
                        on_wait=[waits[-1]], on_update=list(si.on_update))
                newl.append(ins)
            blk.instructions[:] = newl
    return n


_UID = [0]


def blocks(pool, nblk, width, dtype, tag):
    _UID[0] += 1
    t = pool.tile([128, nblk, width], dtype, tag=tag, name=f"{tag}_u{_UID[0]}")
    return [t[:, i, :] for i in range(nblk)]


def build(nbody=1, sim=False):
    nc = bass.Bass("TRN2", target_bir_lowering=False, debug=False,
                   num_devices=1 if sim else NCORES)

    # ---- DRAM I/O ----
    xmT_a = nc.dram_tensor("xmT_a", [D, TA], F32R, kind="ExternalInput").ap()
    xmT_c = nc.dram_tensor("xmT_c", [D, TC], F32R, kind="ExternalInput").ap()
    validk = nc.dram_tensor("validk", [TC], F32, kind="ExternalInput").ap()
    lmask = nc.dram_tensor("lmask", [128, 128], BF16, kind="ExternalInput").ap()
    umask = nc.dram_tensor("umask", [128, 128], BF16, kind="ExternalInput").ap()
    ident = nc.dram_tensor("ident", [128, 128], F32R, kind="ExternalInput").ap()
    wkT = nc.dram_tensor("wkT", [D, D], F32R, kind="ExternalInput").ap()
    wvT = nc.dram_tensor("wvT", [D, D], F32R, kind="ExternalInput").ap()
    wlrT = nc.dram_tensor("wlrT", [D, 1], F32R, kind="ExternalInput").ap()
    w0T = nc.dram_tensor("w0T", [D, D], F32R, kind="ExternalInput").ap()
    w1T = nc.dram_tensor("w1T", [D, D], F32R, kind="ExternalInput").ap()
    w1n = nc.dram_tensor("w1n", [D, D], F32R, kind="ExternalInput").ap()
    wqT = nc.dram_tensor("wqT", [D, D], F32R, kind="ExternalInput").ap()
    swqT = nc.dram_tensor("swqT", [D, D], BF16, kind="ExternalInput").ap()
    swkT = nc.dram_tensor("swkT", [D, D], BF16, kind="ExternalInput").ap()
    swvT = nc.dram_tensor("swvT", [D, D], BF16, kind="ExternalInput").ap()
    swoT = nc.dram_tensor("swoT", [D, D], BF16, kind="ExternalInput").ap()
    out_d = nc.dram_tensor("out", [D, 512], F32, kind="ExternalOutput").ap()

    def load_w(pool, eng, src, name, dtype, width=D):
        """One batched DMA for a [D, width] DRAM weight -> [128, DT, width]."""
        _UID[0] += 1
        t = pool.tile([128, DT, width], dtype, tag=name,
                      name=f"{name}_u{_UID[0]}")
        eng.dma_start(out=t[:, :, :],
                      in_=src.rearrange("(a p) d -> p a d", p=128))
        return [t[:, i, :] for i in range(DT)]

    with tile.TileContext(nc) as tc:
        with (
            tc.tile_pool(name="wpool", bufs=1) as wp,      # persistent
            tc.tile_pool(name="dramp", bufs=1, space="DRAM") as dramp,
        ):
            ident_r = wp.tile([128, 128], F32R, tag="ident_r", name="ident_r")
            nc.scalar.dma_start(out=ident_r, in_=ident)
            ones_b = wp.tile([1, 64], BF16, tag="ones_b", name="ones_b")
            nc.vector.memset(ones_b, 1.0)
            # w_new^T holder (f32r, phase-C stationary); list [l][j]
            wnT_t = wp.tile([128, N_LAYERS, DT, D], F32R, tag="wnT", name="wnT")
            wnT = [[wnT_t[:, l, j, :] for j in range(DT)]
                   for l in range(N_LAYERS)]

            def one_body(body_i):
                with tc.tile_pool(name="cps", bufs=1) as cps:
                    g1_d = dramp.tile([128, DT * D], BF16, tag="g1d",
                                      name=f"g1d_{body_i}")
                    gs1_d = dramp.tile([128, DT * D], BF16, tag="gs1d",
                                       name=f"gs1d_{body_i}")
                    g0_d = dramp.tile([128, DT * D], BF16, tag="g0d",
                                      name=f"g0d_{body_i}")
                    gs0_d = dramp.tile([128, DT * D], BF16, tag="gs0d",
                                       name=f"gs0d_{body_i}")
                    sband = dramp.tile([1, 640], F32, tag="sband",
                                       name=f"sband_{body_i}")

                    # ================= PHASE A =================
                    with (
                        tc.tile_pool(name="apool", bufs=4) as ap,
                        tc.tile_pool(name="apers", bufs=1) as aps,
                        tc.tile_pool(name="psA", bufs=4, space="PSUM") as psA,
                        tc.tile_pool(name="psTr", bufs=2, space="PSUM") as psTr,
                        tc.tile_pool(name="psDw", bufs=2, space="PSUM") as psDw,
                    ):
                        # kproj needs xa+wkT: one on each queue so both land
                        # ~4 us after issue; w0T rides right behind wkT
                        xa = load_w(aps, nc.sync, xmT_a, "xa", F32R, width=TA)
                        wlrT_r = aps.tile([128, DT, 1], F32R, tag="wlrT_r",
                                          name="wlrT_r")
                        nc.scalar.dma_start(
                            out=wlrT_r[:, :, :],
                            in_=wlrT.rearrange("(a p) b -> p a b", p=128))
                        wkT_r = load_w(aps, nc.scalar, wkT, "wkT_r", F32R)
                        w0T_r = load_w(aps, nc.scalar, w0T, "w0T_r", F32R)
                        wvT_r = load_w(aps, nc.sync, wvT, "wvT_r", F32R)
                        w1T_r = load_w(aps, nc.gpsimd, w1T, "w1T_r", F32R)
                        w1n_r = load_w(aps, nc.gpsimd, w1n, "w1n_r", F32R)

                        # ---- phase C inputs stream in on the gpsimd queue,
                        # behind the phase-A gpsimd loads ----
                        xc = load_w(cps, nc.gpsimd, xmT_c, "xc", F32R,
                                    width=TC)
                        wqT_r = load_w(cps, nc.gpsimd, wqT, "wqT_r", F32R)
                        swqT_r = load_w(cps, nc.gpsimd, swqT, "swqT_r", BF16)
                        swkT_r = load_w(cps, nc.gpsimd, swkT, "swkT_r", BF16)
                        swvT_r = load_w(cps, nc.gpsimd, swvT, "swvT_r", BF16)
                        swoT_b = load_w(cps, nc.gpsimd, swoT, "swoT_b", BF16)
                        lmask_b = cps.tile([128, 128], BF16, tag="lmask_b",
                                           name="lmask_b")
                        nc.gpsimd.dma_start(out=lmask_b, in_=lmask)
                        umask_b = cps.tile([128, 128], BF16, tag="umask_b",
                                           name="umask_b")
                        nc.gpsimd.dma_start(out=umask_b, in_=umask)
                        vald = cps.tile([128, 8], F32, tag="vald", name="vald")
                        nc.gpsimd.dma_start(
                            out=vald,
                            in_=validk.rearrange("(c p) -> p c", p=128))

                        def mmT(wtiles, rhs_tiles, name, evac, halves=(0, 1)):
                            for hf, (off, w) in ((i, HALVES[i])
                                                 for i in halves):
                                pss = []
                                for do in range(DT):
                                    ps = psA.tile([128, 264], F32, tag="Amm",
                                                  name=f"{name}_ps{do}_{hf}")
                                    for ki in range(DT):
                                        nc.tensor.matmul(
                                            ps,
                                            wtiles[ki][:, 128 * do:128 * (do + 1)],
                                            rhs_tiles[ki][:, off:off + w],
                                            start=(ki == 0), stop=(ki == DT - 1))
                                    pss.append(ps)
                                evac(off, w, pss)

                        def transpose_into(dst, src, scale_s, name):
                            for tt in range(NTT):
                                tw = TTW[tt]
                                c0 = 128 * tt
                                for do in range(DT):
                                    pt = psTr.tile([128, 128], F32R, tag="Atr",
                                                   name=f"tr_{name}_{tt}_{do}")
                                    nc.tensor.transpose(
                                        pt[0:tw, :],
                                        src[do][:, c0:c0 + tw], ident_r)
                                    dsl = dst[tt][0:tw, 128 * do:128 * (do + 1)]
                                    if scale_s:
                                        nc.vector.tensor_scalar(
                                            dsl, pt[0:tw, :],
                                            s_td_t[0:tw, tt:tt + 1], None,
                                            OP.mult)
                                    else:
                                        nc.scalar.copy(dsl, pt[0:tw, :])

                        # k projection
                        kT = blocks(aps, DT, TA, F32R, "kT")
                        mmT(wkT_r, xa, "kproj",
                            lambda off, w, pss: [nc.scalar.copy(
                                kT[do][:, off:off + w], pss[do])
                                for do in range(DT)])

                        # alr: row [1, TA] then DRAM round-trip to [128, 5]
                        srow = aps.tile([1, 640], F32, tag="srow", name="srow")
                        nc.vector.memset(srow[:, TA:640], 0.0)
                        for hf, (off, w) in enumerate(HALVES):
                            pa = psA.tile([1, 264], F32, tag="Amm",
                                          name=f"alr{hf}")
                            for ki in range(DT):
                                nc.tensor.matmul(pa, wlrT_r[:, ki, :],
                                                 xa[ki][:, off:off + w],
                                                 start=(ki == 0),
                                                 stop=(ki == DT - 1))
                            nc.scalar.activation(srow[:, off:off + w], pa,
                                                 AF.Sigmoid)
                        nc.vector.tensor_scalar_mul(srow[:, 0:TA],
                                                    srow[:, 0:TA],
                                                    2.0 * MAX_ALR / D)
                        nc.sync.dma_start(out=sband, in_=srow)
                        s_td_t = aps.tile([128, NTT], F32, tag="s_td",
                                          name="s_td")
                        nc.sync.dma_start(
                            out=s_td_t,
                            in_=sband.opt().rearrange("a (c p) -> (a p) c",
                                                      p=128))

                        # z0; x1 = k + silu(z0); d0
                        x1T = blocks(aps, DT, TA, F32R, "x1T")
                        d0T = blocks(aps, DT, TA, BF16, "d0T")

                        def z0_evac(off, w, pss):
                            # silu/dsilu batch order flips per half so the
                            # scalar ACT table switches once, not twice
                            sils = []

                            def do_sils():
                                for do in range(DT):
                                    sil = ap.tile([128, 264], F32, tag="silA",
                                                  name=f"sil0_{do}_{off}")
                                    nc.scalar.activation(sil, pss[do], AF.Silu)
                                    sils.append(sil)

                            def do_dsils():
                                for do in range(DT):
                                    nc.scalar.activation(
                                        d0T[do][:, off:off + w],
                                        pss[do], AF.Derivative_silu)

                            do_sils(), do_dsils()
                            for do in range(DT):
                                nc.vector.tensor_tensor(
                                    x1T[do][:, off:off + w],
                                    kT[do][:, off:off + w], sils[do], OP.add)
                        mmT(w0T_r, kT, "z0", z0_evac)

                        # k token-major transposes pulled off the dW0 tail —
                        # kT is final after kproj, so run these early in PE
                        # load-wait gaps
                        k_td = blocks(aps, NTT, D, BF16, "k_td")
                        transpose_into(k_td, kT, False, "k")

                        # v projection interleaved with z1 per half;
                        # dx2 pre-filled with (x1 - v) so the z1 evac chain
                        # is 2 ops: dx2 += silu(z1); dz1 = dx2*d1
                        vT = blocks(aps, DT, TA, BF16, "vT")
                        dz1T = blocks(aps, DT, TA, F32R, "dz1T")
                        dx2T = blocks(aps, DT, TA, BF16, "dx2T")

                        def z1_evac(off, w, pss):
                            sils = []
                            d1s = []

                            def do_sils():
                                for do in range(DT):
                                    sil = ap.tile([128, 264], F32, tag="silA",
                                                  name=f"sil1_{do}_{off}")
                                    nc.scalar.activation(sil, pss[do], AF.Silu)
                                    sils.append(sil)

                            def do_d1s():
                                for do in range(DT):
                                    d1 = ap.tile([128, 264], F32, tag="d1A",
                                                 name=f"d1_{do}_{off}")
                                    nc.scalar.activation(
                                        d1, pss[do], AF.Derivative_silu)
                                    d1s.append(d1)

                            do_sils(), do_d1s()
                            for do in range(DT):
                                nc.vector.tensor_tensor(
                                    dx2T[do][:, off:off + w],
                                    dx2T[do][:, off:off + w], sils[do],
                                    OP.add)
                                nc.vector.tensor_tensor(
                                    dz1T[do][:, off:off + w],
                                    dx2T[do][:, off:off + w], d1s[do], OP.mult)

                        for hf, (off, w) in enumerate(HALVES):
                            mmT(wvT_r, xa, f"vproj{hf}",
                                lambda off, w, pss: [nc.vector.tensor_copy(
                                    vT[do][:, off:off + w], pss[do])
                                    for do in range(DT)],
                                halves=(hf,))
                            for do in range(DT):
                                nc.vector.tensor_tensor(
                                    dx2T[do][:, off:off + w],
                                    x1T[do][:, off:off + w],
                                    vT[do][:, off:off + w], OP.subtract)
                            mmT(w1T_r, x1T, f"z1{hf}", z1_evac, halves=(hf,))

                        # ---- PE transposes into token-major [t, d] (bf16) ----
                        x1_td = blocks(aps, NTT, D, BF16, "x1_td")
                        sdz1_td = blocks(aps, NTT, D, BF16, "sdz1_td")
                        sdz0_td = blocks(aps, NTT, D, BF16, "sdz0_td")

                        def dw_layer(x_td, z_td, g_d, lname):
                            gsb = aps.tile([128, DT * D], BF16,
                                           tag=f"gsb_{lname}",
                                           name=f"gsb_{lname}")
                            for j in range(DT):
                                pdw = psDw.tile([128, D], F32, tag="Adw",
                                                name=f"dw_ps{lname}_{j}")
                                for tt in range(NTT):
                                    tw = TTW[tt]
                                    nc.tensor.matmul(
                                        pdw,
                                        x_td[tt][0:tw, 128 * j:128 * (j + 1)],
                                        z_td[tt][0:tw, :],
                                        start=(tt == 0), stop=(tt == NTT - 1))
                                nc.vector.tensor_copy(
                                    gsb[:, j * D:(j + 1) * D], pdw)
                            nc.sync.dma_start(out=g_d, in_=gsb)

                        def all_reduce(g_d, gs_d):
                            if sim:
                                nc.gpsimd.dma_start(out=gs_d, in_=g_d)
                            else:
                                nc.gpsimd.collective_compute(
                                    "AllReduce", OP.add,
                                    replica_groups=[list(range(NCORES))],
                                    ins=[g_d.opt()], outs=[gs_d.opt()])

                        # dW1 path first, then its AllReduce overlaps the
                        # dW0 backward tail.
                        transpose_into(x1_td, x1T, False, "x1")
                        transpose_into(sdz1_td, dz1T, True, "dz1")
                        dw_layer(x1_td, sdz1_td, g1_d, "l1")
                        all_reduce(g1_d, gs1_d)

                        # u = (dz1 @ W1)^T; dx1 = dx2 + u; dz0 = dx1*d0
                        dz0T = blocks(aps, DT, TA, F32R, "dz0T")

                        def u_evac(off, w, pss):
                            for do in range(DT):
                                dx1 = ap.tile([128, 264], F32R, tag="dx1A",
                                              name=f"dx1_{do}_{off}")
                                nc.vector.tensor_tensor(
                                    dx1, dx2T[do][:, off:off + w], pss[do],
                                    OP.add)
                                nc.vector.tensor_tensor(
                                    dz0T[do][:, off:off + w], dx1,
                                    d0T[do][:, off:off + w], OP.mult)
                        mmT(w1n_r, dz1T, "u", u_evac)

                        transpose_into(sdz0_td, dz0T, True, "dz0")
                        dw_layer(k_td, sdz0_td, g0_d, "l0")
                        all_reduce(g0_d, gs0_d)

                        # prefill wnT = W_l^T * (1 - LR*WD); emitted after the
                        # last critical vector op so it rides the CC window
                        c1 = 1.0 - LR * WD
                        for l, wsrc in enumerate((w0T_r, w1T_r)):
                            for i in range(DT):
                                nc.vector.tensor_scalar_mul(
                                    wnT[l][i], wsrc[i], c1)

                    # ============ OVERLAP + PHASE C ============
                    with (
                        tc.tile_pool(name="cpc", bufs=1) as cpc,
                        tc.tile_pool(name="cpool", bufs=3) as cp,
                        tc.tile_pool(name="cpb", bufs=17) as cpb,
                    ):
                        with tc.tile_pool(name="psC", bufs=3,
                                          space="PSUM") as psC:
                            def mmC(wtiles, rhs_tiles, name, out_cb,
                                    width=TC, roff=0):
                                for off in range(0, width, 512):
                                    for do in range(DT):
                                        ps = psC.tile(
                                            [128, 512], F32, tag="Cmm",
                                            name=f"{name}_ps{do}_{off}")
                                        for ki in range(DT):
                                            nc.tensor.matmul(
                                                ps,
                                                wtiles[ki][:, 128 * do:
                                                           128 * (do + 1)],
                                                rhs_tiles[ki][:, roff + off:
                                                              roff + off + 512],
                                                start=(ki == 0),
                                                stop=(ki == DT - 1))
                                        out_cb(do, off, ps)

                            # q projection overlaps the dW0 AllReduce
                            qT = blocks(cpc, DT, TC, F32R, "qT")
                            mmC(wqT_r, xc, "q",
                                lambda do, off, ps: nc.scalar.copy(
                                    qT[do][:, off:off + 512], ps))

                            # v65 ones columns memset during the CC window
                            v65 = blocks(cpc, 8, H * 65, BF16, "v65")
                            for kt in range(8):
                                v3m = v65[kt].rearrange("p (h c) -> p h c",
                                                        c=65)
                                nc.vector.memset(v3m[:, :, 64:65], 1.0)

                            # weight updates: layer 0 first (unblocks l0);
                            # halves pipelined so l0's first wnT block is
                            # ready before the full sign finishes
                            for l, gs_d in ((0, gs0_d), (1, gs1_d)):
                                gsum = cp.tile([128, DT * D], BF16, tag="gsum",
                                               name=f"gsum{l}")
                                nc.sync.dma_start(out=gsum, in_=gs_d)
                                for hh in range(2):
                                    cw = DT * D // 2
                                    sgn = cp.tile([128, cw], BF16, tag="sgn",
                                                  name=f"sgn{l}_{hh}")
                                    nc.scalar.activation(
                                        sgn, gsum[:, hh * cw:(hh + 1) * cw],
                                        AF.Sign)
                                    wl = wnT_t[:, l, 2 * hh:2 * hh + 2, :]\
                                        .rearrange("p a d -> p (a d)")
                                    nc.vector.scalar_tensor_tensor(
                                        wl, sgn, -LR, wl, OP.mult, OP.add)

                            r0T = blocks(cpc, DT, TC, F32R, "r0T")

                            def l0_out(do, off, ps):
                                sil = cp.tile([128, 512], F32, tag="silC",
                                              name=f"l0s{do}_{off}")
                                nc.scalar.activation(sil, ps, AF.Silu)
                                nc.vector.tensor_tensor(
                                    r0T[do][:, off:off + 512],
                                    qT[do][:, off:off + 512], sil, OP.add)
                            mmC(wnT[0], qT, "l0", l0_out)

                            rT = blocks(cpc, DT, TC, BF16, "rT")

                            def l1_out(do, off, ps):
                                sil = cp.tile([128, 512], F32, tag="silC",
                                              name=f"l1s{do}_{off}")
                                nc.scalar.activation(sil, ps, AF.Silu)
                                nc.vector.tensor_tensor(
                                    rT[do][:, off:off + 512],
                                    r0T[do][:, off:off + 512], sil, OP.add)
                            mmC(wnT[1], r0T, "l1", l1_out)

                            kTb = blocks(cpc, DT, TC, BF16, "kTb")
                            mmC(swkT_r, rT, "sk",
                                lambda do, off, ps: nc.scalar.copy(
                                    kTb[do][:, off:off + 512], ps))
                            qTb = blocks(cpc, DT, 512, BF16, "qTb")
                            mmC(swqT_r, rT, "sq",
                                lambda do, off, ps: nc.scalar.copy(
                                    qTb[do], ps),
                                width=512, roff=512)

                            # v token-major into the interleaved-ones layout
                            for kt in range(8):
                                pv = psC.tile([128, 512], F32, tag="Cmm",
                                              name=f"v_ps{kt}")
                                for ki in range(DT):
                                    nc.tensor.matmul(
                                        pv, rT[ki][:, 128 * kt:128 * (kt + 1)],
                                        swvT_r[ki], start=(ki == 0),
                                        stop=(ki == DT - 1))
                                v3 = v65[kt].rearrange("p (h c) -> p h c", c=65)
                                nc.vector.tensor_copy(
                                    v3[:, :, 0:64],
                                    pv.rearrange("p (h c) -> p h c", c=64))

                        # ---- attention per head ----
                        oTb = blocks(cpc, DT, 512, BF16, "oTb")
                        with (
                            tc.tile_pool(name="psS", bufs=3,
                                         space="PSUM") as psS,
                            tc.tile_pool(name="psAv", bufs=4,
                                         space="PSUM") as psAv,
                            tc.tile_pool(name="psB", bufs=1,
                                         space="PSUM") as psB,
                        ):
                            # Software pipeline over heads: head h's score
                            # burst runs while head h-1's AV burst drains.
                            # av tiles stay in PSUM (groups of 4 heads);
                            # denominator reciprocals run batched (4 heads
                            # per DVE op — free-dim length drives DVE cost,
                            # not partition count).
                            # head h's denominator parks at partition
                            # 32*(h%4) of group h//4 (DVE partition bases
                            # must be multiples of 32)
                            denG = cpc.tile([128, 2, 512], F32, tag="denG",
                                            name="denG")
                            recG = cpc.tile([128, 2, 512], F32, tag="recG",
                                            name="recG")
                            nc.vector.memset(denG, 1.0)

                            def sc_block(h):
                                th, base = h // 2, 64 * (h % 2)
                                pbfs = []
                                for kt in range(8):
                                    qlo = 128 * max(0, kt - 4)
                                    qhi = min(512, 128 * (kt + 1))
                                    wdt = qhi - qlo
                                    sc = psS.tile([128, 512], F32, tag="Sc",
                                                  name=f"sc{h}_{kt}")
                                    nc.tensor.matmul(
                                        sc[:, 0:wdt],
                                        kTb[th][base:base + 64,
                                                128 * kt:128 * (kt + 1)],
                                        qTb[th][base:base + 64, qlo:qhi],
                                        start=True, stop=True,
                                        tile_position=(base, 0))
                                    pbf = cpb.tile([128, 512], BF16, tag="Pbf",
                                                   name=f"p{h}_{kt}")
                                    nc.scalar.activation(
                                        pbf[:, 0:wdt], sc[:, 0:wdt], AF.Exp,
                                        scale=0.125, bias=vald[:, kt:kt + 1])
                                    if kt <= 3:
                                        nc.vector.tensor_tensor(
                                            pbf[:, wdt - 128:wdt],
                                            pbf[:, wdt - 128:wdt],
                                            lmask_b, OP.mult)
                                    if kt >= 4:
                                        nc.vector.tensor_tensor(
                                            pbf[:, 0:128], pbf[:, 0:128],
                                            umask_b, OP.mult)
                                    pbfs.append((pbf, qlo, qhi, wdt))
                                return pbfs

                            avps = {}

                            def av_block(h, pbfs):
                                av = psAv.tile([65, 512], F32, tag="Av",
                                               name=f"av{h}")
                                avps[h] = av
                                for kt, (pbf, qlo, qhi, wdt) in enumerate(pbfs):
                                    nc.tensor.matmul(
                                        av[:, qlo:qhi],
                                        v65[kt][:, 65 * h:65 * h + 65],
                                        pbf[:, 0:wdt],
                                        start=(kt == 0), stop=(kt == 7))
                                g, m = divmod(h, 4)
                                nc.vector.tensor_copy(
                                    denG[32 * m:32 * m + 1, g, :],
                                    av[64:65, :])

                            def recip_batch(g, lo=0, hi=128):
                                nc.vector.reciprocal(recG[lo:hi, g, :],
                                                     denG[lo:hi, g, :])

                            def fin_block(h):
                                th, base = h // 2, 64 * (h % 2)
                                g, m = divmod(h, 4)
                                rdb = cp.tile([1, 512], BF16, tag="rdb",
                                              name=f"rdb{h}")
                                nc.vector.tensor_copy(
                                    rdb, recG[32 * m:32 * m + 1, g, :])
                                pb = psB.tile([64, 512], F32, tag="Bc",
                                              name=f"pb{h}")
                                nc.tensor.matmul(pb, ones_b, rdb,
                                                 start=True, stop=True)
                                rbc = cp.tile([64, 512], BF16, tag="rbc",
                                              name=f"rbc{h}")
                                nc.vector.tensor_copy(rbc, pb)
                                nc.vector.tensor_tensor(
                                    oTb[th][base:base + 64, :],
                                    avps.pop(h)[0:64, :], rbc, OP.mult)

                            prev_pbfs = None
                            for h in range(H):
                                pbfs = sc_block(h)
                                if h == 5:
                                    for hf in range(4):
                                        fin_block(hf)
                                if h == 7:
                                    fin_block(4)
                                    fin_block(5)
                                if prev_pbfs is not None:
                                    av_block(h - 1, prev_pbfs)
                                if h == 4:
                                    recip_batch(0)
                                if h == 6:
                                    recip_batch(1, 0, 64)   # heads 4,5
                                prev_pbfs = pbfs
                            av_block(H - 1, prev_pbfs)
                            recip_batch(1, 64, 128)         # heads 6,7
                            fin_block(6)
                            fin_block(7)

                            # output projection + store
                            for do in range(DT):
                                po = psS.tile([128, 512], F32, tag="Sc",
                                              name=f"o_ps{do}")
                                for ki in range(DT):
                                    nc.tensor.matmul(
                                        po,
                                        swoT_b[ki][:, 128 * do:128 * (do + 1)],
                                        oTb[ki], start=(ki == 0),
                                        stop=(ki == DT - 1))
                                ofin = cp.tile([128, 512], F32, tag="ofin",
                                               name=f"ofin{do}")
                                nc.scalar.copy(ofin, po)
                                nc.sync.dma_start(
                                    out=out_d[128 * do:128 * (do + 1), :],
                                    in_=ofin)

            for _bi in range(nbody):
                one_body(_bi)
    return nc


_CACHE = {}


def _get_nc(nbody=1):
    key = f"nc{nbody}"
    if key not in _CACHE:
        nc = build(nbody)
        split_waits(nc)
        _CACHE[key] = nc
    return _CACHE[key]


def prepare_in_maps(x, meta_memory, lmm_w, w_q, w_k, w_v, w_lr,
                    swa_wq, swa_wk, swa_wv, swa_wo):
    x = np.asarray(x, np.float32)
    meta_memory = np.asarray(meta_memory, np.float32)
    lmm_w = np.asarray(lmm_w, np.float32)
    xm = np.concatenate(
        [np.broadcast_to(meta_memory, (B,) + meta_memory.shape), x], axis=1)
    xf = xm.reshape(NTOK, D)

    import ml_dtypes
    bfd = ml_dtypes.bfloat16
    tri = np.arange(128)
    lmask_np = (tri[None, :] < tri[:, None]).astype(bfd)   # qj < ki
    umask_np = (tri[None, :] >= tri[:, None]).astype(bfd)  # qj >= ki
    ident_np = np.eye(128, dtype=np.float32)

    common = {
        "lmask": lmask_np, "umask": umask_np, "ident": ident_np,
        "wkT": np.ascontiguousarray(np.asarray(w_k, np.float32).T),
        "wvT": np.ascontiguousarray(np.asarray(w_v, np.float32).T),
        "wlrT": np.ascontiguousarray(np.asarray(w_lr, np.float32).T),
        "w0T": np.ascontiguousarray(lmm_w[0].T),
        "w1T": np.ascontiguousarray(lmm_w[1].T),
        "w1n": np.ascontiguousarray(lmm_w[1]),
        "wqT": np.ascontiguousarray(np.asarray(w_q, np.float32).T),
        "swqT": np.ascontiguousarray(np.asarray(swa_wq, np.float32).T).astype(bfd),
        "swkT": np.ascontiguousarray(np.asarray(swa_wk, np.float32).T).astype(bfd),
        "swvT": np.ascontiguousarray(np.asarray(swa_wv, np.float32).T).astype(bfd),
        "swoT": np.ascontiguousarray(np.asarray(swa_wo, np.float32).T).astype(bfd),
    }
    in_maps = []
    for c in range(NCORES):
        xa = np.ascontiguousarray(xf[TA * c:TA * (c + 1)].T)
        b, r = c // 4, c % 4
        t1 = M + 512 * (r + 1)
        lo = max(t1 - TC, 0)
        pad = TC - (t1 - lo)
        xcm = np.zeros((D, TC), np.float32)
        xcm[:, pad:] = xm[b, lo:t1].T
        vk = np.full(TC, -30.0, np.float32)
        vk[pad:] = 0.0
        mcore = dict(common)
        mcore["xmT_a"] = xa
        mcore["xmT_c"] = xcm
        mcore["validk"] = vk
        in_maps.append(mcore)
    return in_maps


def run_on_device(in_maps, nbody=1):
    nc = _get_nc(nbody)
    return bass_utils.run_bass_kernel_spmd(nc, in_maps,
                                           core_ids=list(range(NCORES)))


def kernel(**inputs):
    in_maps = prepare_in_maps(**inputs)
    res = run_on_device(in_maps)
    out = np.empty((B, S, D), np.float32)
    for c in range(NCORES):
        b, r = c // 4, c % 4
        out[b, 512 * r:512 * (r + 1), :] = res.results[c]["out"].T
    return out
